# revision 52
# baseline (speedup 1.0000x reference)
"""Causal multi-head attention (B=4, S=2048, D=1024, H=16, hd=64) on 8
Trainium2 NeuronCores.

Sharding: batch (4-way) x head-group (2-way). Core c handles batch c//2 and
heads [8*(c%2), 8*(c%2)+8). Each core computes its heads' contribution to the
output projection; the host sums the two partials per batch and adds bo.

v2 design (all matmul operands bf16, fp32 PSUM accumulate):
  - Fully software-pipelined single pass over 4 query windows of 512.
    Window w runs flash-style attention (transposed layout: scoresT =
    K_chunk @ Q^T, row-group pairs concurrent on the PE) while the
    K/Q projections for window w+1, the V projections carried from the
    previous window, and the output projection for window w-2 are woven
    between attention chunk batches, so the PE never idles long enough
    for HAM to re-throttle. Emission uses pull semantics (`need`) so a
    consumer is never emitted before its producer (Tile only tracks
    backward dependencies; emitting a read first is a silent race).
  - Causal: block-skipping plus affine_select triangular masking on
    diagonal 128x128 blocks (gpsimd, both par halves in one op).
  - Softmax denominator via fused ones-column in V (o2 row 64 = Z).
    Normalization is off the PE critical path: o2 is evicted to SBUF
    immediately (PSUM freed), the Z row is DMA-scattered onto 16
    partitions so one exact DVE reciprocal covers it in ~0.55us, then
    gather + partition_broadcast + two muls produce bf16 ao tiles.
    (reciprocal_approx_fast computes garbage on this HW; vector
    reciprocal on a [1,1024] single-lane AP costs 8.5us - avoid both.)
  - Tail: dummy bf16 matmuls reading the last oz keep HAM at full
    clock across the final norm chain; output DMAs and evictions
    alternate queues/engines so the drain pipelines.
  - PSUM budget exactly 8 banks: 2x s2 (2 banks each), 1x o2 (2 banks),
    2x projection accumulators (1 bank each).
"""
import numpy as np
import ml_dtypes

import concourse.mybir as mybir
from concourse import bacc
from concourse.tile import TileContext
from concourse.bass_utils import run_bass_kernel_spmd

FP32 = mybir.dt.float32
BF16 = mybir.dt.bfloat16
EXP = mybir.ActivationFunctionType.Exp

B, S, D = 4, 2048, 1024
H, HD = 16, 64
NCORES = 8
HPG = 8              # heads per group (per core)
GD = HPG * HD        # 512: group head-dim width
W = 512              # query window
NW = S // W          # 4
KCH = 128            # key chunk
NKC = S // KCH       # 16
DC = 128             # D contraction chunk
NDC = D // DC        # 8
SCALE = 1.0 / 8.0    # 1/sqrt(hd)

_CACHE = {}


def _build_program():
    nc = bacc.Bacc("TRN2", target_bir_lowering=False, debug=False,
                   num_devices=NCORES)

    xT = nc.dram_tensor("xT", [D, S], BF16, kind="ExternalInput").ap()
    wq = nc.dram_tensor("wq", [D, GD], BF16, kind="ExternalInput").ap()
    wk = nc.dram_tensor("wk", [D, GD], BF16, kind="ExternalInput").ap()
    wv = nc.dram_tensor("wv", [D, GD], BF16, kind="ExternalInput").ap()
    wo = nc.dram_tensor("wo", [GD, D], BF16, kind="ExternalInput").ap()
    bq2 = nc.dram_tensor("bq2", [128, 4], FP32, kind="ExternalInput").ap()
    bk2 = nc.dram_tensor("bk2", [128, 4], FP32, kind="ExternalInput").ap()
    out = nc.dram_tensor("out", [S, D], FP32, kind="ExternalOutput").ap()

    with TileContext(nc) as tc:
        with (
            tc.tile_pool(name="cst", bufs=1) as cst_pool,
            tc.tile_pool(name="wts", bufs=3) as wts_pool,
            tc.tile_pool(name="wo", bufs=1) as wo_pool,
            tc.tile_pool(name="xt", bufs=2) as xt_pool,
            tc.tile_pool(name="kt", bufs=16) as kt_pool,
            tc.tile_pool(name="vst", bufs=16) as v_pool,
            tc.tile_pool(name="qt", bufs=8) as qt_pool,
            tc.tile_pool(name="et", bufs=6) as et_pool,
            tc.tile_pool(name="oz", bufs=5) as oz_pool,
            tc.tile_pool(name="zz", bufs=4) as zz_pool,
            tc.tile_pool(name="zt", bufs=3) as zt_pool,
            tc.tile_pool(name="ao", bufs=12) as ao_pool,
            tc.tile_pool(name="ob", bufs=6) as out_pool,
            tc.tile_pool(name="ps2", bufs=2, space="PSUM") as ps_s2,
            tc.tile_pool(name="po2", bufs=1, space="PSUM") as ps_o2,
            tc.tile_pool(name="ppj", bufs=2, space="PSUM") as ps_pj,
        ):
            # ---- constants: biases ----
            bq_t = cst_pool.tile([128, 4], FP32, tag="bq")
            bk_t = cst_pool.tile([128, 4], FP32, tag="bk")
            nc.sync.dma_start(out=bq_t[:], in_=bq2[:])
            nc.sync.dma_start(out=bk_t[:], in_=bk2[:])

            # ---- persistent SBUF tensors ----
            # kt[hp][w]: [128 (2 par x 64 hd), 512 keys] bf16
            kt_t = [[kt_pool.tile([128, W], BF16, tag="kt", name=f"kt{h}_{w}")
                     for w in range(NW)] for h in range(4)]
            # v[kc]: [128 keys, 8 heads x 65] bf16 (col 64 of each head = 1.0)
            v_t = [v_pool.tile([128, 8 * 65], BF16, tag="v", name=f"v{i}")
                   for i in range(NKC)]
            for kc in range(NKC):
                ones_ap = v_t[kc][:].rearrange(
                    "p (h e) -> p h e", e=65)[:, :, 64:65]
                nc.gpsimd.memset(ones_ap, 1.0)

            # ---- weight tiles (all persistent, bf16) ----
            # Contiguous per-chunk DMAs (strided big-AP DMAs have
            # pathological dispatch cost), distributed across the three
            # DMA-capable queues so dispatch serialization doesn't gate
            # the preamble: wk on gpsimd, wq/wv on scalar (idle during
            # the preamble), wo on sync.
            def load_w(pool, dram, n, pfx, eng):
                cols = dram.shape[1]
                big = pool.tile([128, n * cols], BF16, tag="w", name=pfx)
                for i in range(n):
                    eng.dma_start(
                        out=big[:, i * cols:(i + 1) * cols],
                        in_=dram[i * 128:(i + 1) * 128, :])
                return [big[:, i * cols:(i + 1) * cols] for i in range(n)]
            wk_t = load_w(wts_pool, wk, NDC, "wk", nc.gpsimd)
            wq_t = load_w(wts_pool, wq, NDC, "wq", nc.scalar)
            wv_t = load_w(wts_pool, wv, NDC, "wv", nc.scalar)
            wo_t = load_w(wo_pool, wo, 4, "wo", nc.sync)

            # ---- xt loading (contiguous per-chunk DMAs, sync queue) ----
            def load_xt(w):
                big = xt_pool.tile([128, NDC * W], BF16, tag="xt",
                                   name=f"xt{w}")
                for dc in range(NDC):
                    nc.sync.dma_start(
                        out=big[:, dc * W:(dc + 1) * W],
                        in_=xT[dc * DC:(dc + 1) * DC, w * W:(w + 1) * W])
                return [big[:, dc * W:(dc + 1) * W] for dc in range(NDC)]

            qt_by_w = {}

            # ---- projection micro-step builders (for weaving) ----
            # Each "step" is a closure emitting ~2 matmuls; steps of one
            # group share a PSUM tile via a cell.
            def kq_group_steps(wdst, hp, xts, which):
                # steps are (resource_label_or_None, fn): the label marks
                # the step whose emission completes that resource.
                wt = wk_t if which == "k" else wq_t
                cell = {}

                def mk(dc0):
                    def step():
                        if dc0 == 0:
                            cell["ps"] = ps_pj.tile([128, W], FP32, tag="pj",
                                                    name=f"{which}ps")
                        for dc in (dc0, dc0 + 1):
                            nc.tensor.matmul(
                                cell["ps"][:],
                                wt[dc][:, hp * 128:(hp + 1) * 128],
                                xts[dc][:], start=(dc == 0),
                                stop=(dc == NDC - 1))
                        if dc0 == NDC - 2:
                            if which == "k":
                                nc.vector.tensor_scalar_add(
                                    kt_t[hp][wdst][:], cell["ps"][:],
                                    bk_t[:, hp:hp + 1])
                            else:
                                nc.vector.tensor_scalar_add(
                                    qt_by_w[wdst][hp][:], cell["ps"][:],
                                    bq_t[:, hp:hp + 1])
                    return ((which, hp, wdst) if dc0 == NDC - 2 else None,
                            step)
                return [mk(d) for d in range(0, NDC, 2)]

            def v_group_steps(wdst, sc, xts):
                cell = {}

                def mk(dc0):
                    def step():
                        if dc0 == 0:
                            cell["ps"] = ps_pj.tile([128, W], FP32, tag="pj",
                                                    name="vps")
                        for dc in (dc0, dc0 + 1):
                            nc.tensor.matmul(
                                cell["ps"][:],
                                xts[dc][:, sc * 128:(sc + 1) * 128],
                                wv_t[dc][:], start=(dc == 0),
                                stop=(dc == NDC - 1))
                        if dc0 == NDC - 2:
                            dst = v_t[wdst * 4 + sc][:].rearrange(
                                "p (h e) -> p h e", e=65)[:, :, 0:64]
                            src = cell["ps"][:].rearrange(
                                "p (h e) -> p h e", e=64)
                            nc.vector.tensor_copy(dst, src)
                    return (("v", wdst * 4 + sc) if dc0 == NDC - 2 else None,
                            step)
                return [mk(d) for d in range(0, NDC, 2)]

            def o_group_steps(wsrc, qs, dcol, aos):
                cell = {}

                def mk(hc0):
                    def step():
                        if hc0 == 0:
                            cell["ps"] = ps_pj.tile([128, W], FP32, tag="pj",
                                                    name="ops")
                        for hc in (hc0, hc0 + 1):
                            nc.tensor.matmul(
                                cell["ps"][:],
                                aos[hc][:, qs * 128:(qs + 1) * 128],
                                wo_t[hc][:, dcol * 512:(dcol + 1) * 512],
                                start=(hc == 0), stop=(hc == 3))
                        if hc0 == 2:
                            ot = out_pool.tile([128, W], FP32, tag="ob")
                            # alternate output DMA queues; in the tail
                            # window (ACT idle) also alternate eviction
                            # engines so evictions and transfers pipeline
                            if (qs + dcol) % 2 == 0:
                                nc.vector.tensor_copy(ot[:], cell["ps"][:])
                                eng = nc.sync
                            else:
                                if wsrc == NW - 1:
                                    nc.scalar.copy(ot[:], cell["ps"][:])
                                else:
                                    nc.vector.tensor_copy(ot[:],
                                                          cell["ps"][:])
                                eng = nc.gpsimd
                            eng.dma_start(
                                out=out[wsrc * W + qs * 128:
                                        wsrc * W + (qs + 1) * 128,
                                        dcol * 512:(dcol + 1) * 512],
                                in_=ot[:])
                    return (None, step)
                return [mk(h) for h in range(0, 4, 2)]

            # ---- attention emitters ----
            def emit_S(w, hp, kcs, qts):
                """Score matmuls + exp (+ diag mask) for a batch of chunks.
                Returns {kc: et_tile}."""
                ets = {}
                for kc in kcs:
                    j = kc - 4 * w
                    lo = max(j, 0) * 128
                    s2 = ps_s2.tile([128, 1024], FP32, tag="s2", name="s2")
                    et = et_pool.tile([128, 1024], BF16, tag="et")
                    for par in range(2):
                        nc.tensor.matmul(
                            s2[:, par * 512 + lo:par * 512 + 512],
                            kt_t[hp][kc // 4][par * 64:(par + 1) * 64,
                                              (kc % 4) * 128:(kc % 4 + 1) * 128],
                            qts[hp][par * 64:(par + 1) * 64, lo:W],
                            start=True, stop=True)
                    if lo == 0:
                        nc.scalar.activation(et[:], s2[:], EXP,
                                             bias=0.0, scale=SCALE)
                    else:
                        sv = s2[:].rearrange("p (two n) -> p two n",
                                             two=2)[:, :, lo:512]
                        ev = et[:].rearrange("p (two n) -> p two n",
                                             two=2)[:, :, lo:512]
                        nc.scalar.activation(ev, sv, EXP,
                                             bias=0.0, scale=SCALE)
                    if j >= 0:
                        seg = et[:].rearrange("p (two n) -> p two n",
                                              two=2)[:, :, lo:lo + 128]
                        nc.gpsimd.affine_select(
                            out=seg, in_=seg,
                            compare_op=mybir.AluOpType.is_ge,
                            fill=0.0, base=0, pattern=[[0, 2], [1, 128]],
                            channel_multiplier=-1)
                    ets[kc] = et
                return ets

            def emit_V(w, hp, o2, kcs, ets, nkc):
                for kc in kcs:
                    j = kc - 4 * w
                    lo = max(j, 0) * 128
                    for par in range(2):
                        h = 2 * hp + par
                        nc.tensor.matmul(
                            o2[0:65, par * 512 + lo:par * 512 + 512],
                            v_t[kc][:, h * 65:(h + 1) * 65],
                            ets[kc][:, par * 512 + lo:par * 512 + 512],
                            start=(kc == 0), stop=(kc == nkc - 1))

            def emit_norm_hp(w, hp, o2, aos):
                # Evict o2 (incl. Z row 64) to SBUF fast, freeing PSUM.
                # Z handling: scatter the [1,1024] Z row into 16 partitions
                # (reciprocal then runs on 16 lanes: ~0.55us vs 8.5us),
                # gather back, broadcast per par half, two muls -> bf16 ao.
                # In the last window the scalar queue dispatches the Z DMAs
                # (ACT is idle there; sync's FIFO is busy with output DMAs).
                zq = nc.scalar if w == NW - 1 else nc.sync
                oz = oz_pool.tile([128, 1024], FP32, tag="oz",
                                  name=f"oz{w}_{hp}")
                nc.vector.tensor_copy(oz[0:65, :], o2[0:65, :])
                zzT = zt_pool.tile([16, 64], FP32, tag="zzT")
                zq.dma_start(
                    out=zzT[:],
                    in_=oz[64:65, :].rearrange("p (f n) -> p f n", f=16))
                zrT = zt_pool.tile([16, 64], FP32, tag="zrT")
                nc.vector.reciprocal(zrT[:], zzT[:])
                zr = zz_pool.tile([1, 1024], FP32, tag="zr")
                zq.dma_start(
                    out=zr[:].rearrange("p (f n) -> p f n", f=16),
                    in_=zrT[:])
                for par in range(2):
                    zb = zz_pool.tile([64, 512], FP32, tag="zb")
                    nc.gpsimd.partition_broadcast(
                        zb[:], zr[0:1, par * 512:(par + 1) * 512])
                    nc.vector.tensor_mul(
                        aos[hp][par * 64:(par + 1) * 64, :],
                        oz[0:64, par * 512:(par + 1) * 512],
                        zb[:])
                return oz

            # ---- minimal preamble: only what window-0/hp0 attention
            # needs up front (K/Q for hp0, all V chunks of window 0);
            # the remaining hp1-3 K/Q projections weave into window 0.
            xts0 = load_xt(0)
            qt_by_w[0] = [qt_pool.tile([128, W], BF16, tag="qt",
                                       name=f"qt0_{h}") for h in range(4)]
            emitted_res = set()
            pre_steps = kq_group_steps(0, 0, xts0, "k")
            pre_steps += kq_group_steps(0, 0, xts0, "q")
            for lab, st in pre_steps:
                st()
                if lab:
                    emitted_res.add(lab)
            w0_rest = []
            for sc in range(4):
                w0_rest += v_group_steps(0, sc, xts0)
            for hp in range(1, 4):
                w0_rest += kq_group_steps(0, hp, xts0, "k")
                w0_rest += kq_group_steps(0, hp, xts0, "q")

            ao_by_w = {}
            vcarry = []
            last_oz = None

            # ---- main loop over query windows ----
            for w in range(NW):
                nkc = 4 * (w + 1)
                qts = qt_by_w[w]
                ao_by_w[w] = [ao_pool.tile([128, W], BF16, tag="ao",
                                           name=f"ao{w}_{h}")
                              for h in range(4)]

                # build this window's proj-step queue; V-projection groups
                # for window w+1 carry over to the FRONT of window w+1's
                # queue (they are needed later than K/Q), relieving the
                # PE-heavy early windows.
                steps = []
                steps += vcarry
                vcarry = []
                if w == 0:
                    steps += w0_rest
                if w + 1 < NW:
                    xts = load_xt(w + 1)
                    qt_by_w[w + 1] = [qt_pool.tile([128, W], BF16, tag="qt",
                                                   name=f"qt{w + 1}_{h}")
                                      for h in range(4)]
                    for hp in range(4):
                        steps += kq_group_steps(w + 1, hp, xts, "k")
                        steps += kq_group_steps(w + 1, hp, xts, "q")
                    for sc in range(4):
                        vcarry += v_group_steps(w + 1, sc, xts)
                # O-projections run two windows late (w-2) except w3,
                # which also absorbs w2's: the late windows are ACT-bound
                # with PE slack, the early ones are PE-bound.
                osrcs = []
                if w >= 2:
                    osrcs.append(w - 2)
                if w == NW - 1:
                    osrcs.append(w - 1)
                for wsrc in osrcs:
                    for qs in range(4):
                        for dcol in range(2):
                            steps += o_group_steps(wsrc, qs, dcol,
                                                   ao_by_w[wsrc])
                # consume steps evenly across attention batches; `need`
                # force-pops until a resource's producer has been emitted
                # (program order must respect producer-before-consumer).
                nbatch = (nkc + 1) // 2 * 4          # batches this window
                sq = list(steps)
                popped = [0]
                bdone = [0]

                def pop_one():
                    lab, st = sq[popped[0]]
                    st()
                    if lab:
                        emitted_res.add(lab)
                    popped[0] += 1

                def pop_steps():
                    bdone[0] += 1
                    want = min(len(sq),
                               (len(sq) * bdone[0]) // nbatch + 2)
                    while popped[0] < want:
                        pop_one()

                def need(lab):
                    while lab not in emitted_res:
                        assert popped[0] < len(sq), f"missing producer {lab}"
                        pop_one()

                for hp in range(4):
                    need(("k", hp, w))
                    need(("q", hp, w))
                    o2 = ps_o2.tile([128, 1024], FP32, tag="o2",
                                    name=f"o2_{w}_{hp}")
                    pending = None
                    for kc0 in range(0, nkc, 2):
                        kcs = list(range(kc0, min(kc0 + 2, nkc)))
                        ets = emit_S(w, hp, kcs, qts)
                        pop_steps()
                        if pending is not None:
                            for kc in pending[0]:
                                need(("v", kc))
                            emit_V(w, hp, o2, pending[0], pending[1], nkc)
                        pending = (kcs, ets)
                    for kc in pending[0]:
                        need(("v", kc))
                    emit_V(w, hp, o2, pending[0], pending[1], nkc)
                    last_oz = emit_norm_hp(w, hp, o2, ao_by_w[w])
                # leftover proj steps (rounding)
                while popped[0] < len(sq):
                    pop_one()

            # ---- tail: output projection for the last window ----
            # Dummy accumulation reading the last oz keeps the PE busy
            # across the hp3 norm chain (scatter/recip/gather/bcast/mul)
            # so HAM stays at full clock for the tail matmuls. The oz
            # data dependency pins these into the gap.
            ozb = oz_pool.tile([128, 1024], BF16, tag="ozb", name="ozb")
            nc.vector.tensor_copy(ozb[0:64, 0:512], last_oz[0:64, 0:512])
            warm = ps_pj.tile([128, W], FP32, tag="pj", name="warm")
            for i in range(8):
                nc.tensor.matmul(warm[0:64, :], ozb[0:64, 0:64],
                                 ozb[0:64, 0:512],
                                 start=(i == 0), stop=(i == 7))
            for qs in range(4):
                for dcol in range(2):
                    for _lab, st in o_group_steps(NW - 1, qs, dcol,
                                                  ao_by_w[NW - 1]):
                        st()

    nc.compile()
    return nc


def _get_program():
    if "nc" not in _CACHE:
        _CACHE["nc"] = _build_program()
    return _CACHE["nc"]


def _install_ntff_hook():
    """The agent image's antenv lacks axon_hooks; shim it and register the
    ctypes NTFF profiling hook so trace=True yields exec_time_ns."""
    import sys, types
    if "antenv.axon_hooks" in sys.modules:
        return
    try:
        import antenv
        mod = types.ModuleType("antenv.axon_hooks")
        _h = [None]
        mod.set_axon_ntff_profile_hook = lambda h: _h.__setitem__(0, h)
        mod.get_axon_ntff_profile_hook = lambda: _h[0]
        sys.modules["antenv.axon_hooks"] = mod
        antenv.axon_hooks = mod
        from trn_agent_boot.trn_boot import _ntff_profile_via_ctypes
        mod.set_axon_ntff_profile_hook(
            _ntff_profile_via_ctypes("/opt/axon/libaxon_pjrt.so"))
    except Exception as e:  # degrade: run without tracing
        print(f"NTFF hook install failed ({e}); tracing disabled")


def _run(inputs, trace=False):
    bf = ml_dtypes.bfloat16
    x = np.asarray(inputs["x"], dtype=np.float32)
    Wq = np.asarray(inputs["Wq"], dtype=np.float32)
    Wk = np.asarray(inputs["Wk"], dtype=np.float32)
    Wv = np.asarray(inputs["Wv"], dtype=np.float32)
    Wo = np.asarray(inputs["Wo"], dtype=np.float32)
    bq = np.asarray(inputs["bq"], dtype=np.float32)
    bk = np.asarray(inputs["bk"], dtype=np.float32)
    bv = np.asarray(inputs["bv"], dtype=np.float32)
    bo = np.asarray(inputs["bo"], dtype=np.float32)

    if trace:
        _install_ntff_hook()
    nc = _get_program()
    in_maps = []
    for c in range(NCORES):
        b, g = divmod(c, 2)
        sl = slice(g * GD, (g + 1) * GD)
        in_maps.append({
            "xT": np.ascontiguousarray(x[b].T).astype(bf),
            "wq": np.ascontiguousarray(Wq[:, sl]).astype(bf),
            "wk": np.ascontiguousarray(Wk[:, sl]).astype(bf),
            "wv": np.ascontiguousarray(Wv[:, sl]).astype(bf),
            "wo": np.ascontiguousarray(Wo[sl, :]).astype(bf),
            "bq2": np.ascontiguousarray(bq[sl].reshape(4, 128).T),
            "bk2": np.ascontiguousarray(bk[sl].reshape(4, 128).T),
        })
    res = run_bass_kernel_spmd(nc, in_maps, list(range(NCORES)), trace=trace)
    outp = np.empty((B, S, D), dtype=np.float32)
    # bv correction: attention rows sum to 1, so x @ Wv + bv contributes
    # attn@V + bv per row; bv flows through Wo as a constant row vector.
    corr = (bv @ Wo + bo).astype(np.float32)
    for b in range(B):
        outp[b] = res.results[2 * b]["out"] + res.results[2 * b + 1]["out"] + corr
    return outp, res


def kernel(**inputs):
    outp, _ = _run(inputs, trace=False)
    return outp


def kernel_traced(**inputs):
    outp, res = _run(inputs, trace=True)
    return outp, res


# revision 53
# speedup vs baseline: 1.0197x; 1.0197x over previous
"""Causal multi-head attention (B=4, S=2048, D=1024, H=16, hd=64) on 8
Trainium2 NeuronCores.

Sharding: batch (4-way) x head-group (2-way). Core c handles batch c//2 and
heads [8*(c%2), 8*(c%2)+8). Each core computes its heads' contribution to the
output projection; the host sums the two partials per batch and adds bo.

v2 design (all matmul operands bf16, fp32 PSUM accumulate):
  - Fully software-pipelined single pass over 4 query windows of 512.
    Window w runs flash-style attention (transposed layout: scoresT =
    K_chunk @ Q^T, row-group pairs concurrent on the PE) while the
    K/Q projections for window w+1, the V projections carried from the
    previous window, and the output projection for window w-2 are woven
    between attention chunk batches, so the PE never idles long enough
    for HAM to re-throttle. Emission uses pull semantics (`need`) so a
    consumer is never emitted before its producer (Tile only tracks
    backward dependencies; emitting a read first is a silent race).
  - Causal: block-skipping plus affine_select triangular masking on
    diagonal 128x128 blocks (gpsimd, both par halves in one op).
  - Softmax denominator via fused ones-column in V (o2 row 64 = Z).
    Normalization is off the PE critical path: o2 is evicted to SBUF
    immediately (PSUM freed), the Z row is DMA-scattered onto 16
    partitions so one exact DVE reciprocal covers it in ~0.55us, then
    gather + partition_broadcast + two muls produce bf16 ao tiles.
    (reciprocal_approx_fast computes garbage on this HW; vector
    reciprocal on a [1,1024] single-lane AP costs 8.5us - avoid both.)
  - Tail: dummy bf16 matmuls reading the last oz keep HAM at full
    clock across the final norm chain; output DMAs and evictions
    alternate queues/engines so the drain pipelines.
  - PSUM budget exactly 8 banks: 2x s2 (2 banks each), 1x o2 (2 banks),
    2x projection accumulators (1 bank each).
"""
import numpy as np
import ml_dtypes

import concourse.mybir as mybir
from concourse import bacc
from concourse.tile import TileContext
from concourse.bass_utils import run_bass_kernel_spmd

FP32 = mybir.dt.float32
BF16 = mybir.dt.bfloat16
EXP = mybir.ActivationFunctionType.Exp

B, S, D = 4, 2048, 1024
H, HD = 16, 64
NCORES = 8
HPG = 8              # heads per group (per core)
GD = HPG * HD        # 512: group head-dim width
W = 512              # query window
NW = S // W          # 4
KCH = 128            # key chunk
NKC = S // KCH       # 16
DC = 128             # D contraction chunk
NDC = D // DC        # 8
SCALE = 1.0 / 8.0    # 1/sqrt(hd)

_CACHE = {}


def _build_program():
    nc = bacc.Bacc("TRN2", target_bir_lowering=False, debug=False,
                   num_devices=NCORES)

    xT = nc.dram_tensor("xT", [D, S], BF16, kind="ExternalInput").ap()
    wq = nc.dram_tensor("wq", [D, GD], BF16, kind="ExternalInput").ap()
    wk = nc.dram_tensor("wk", [D, GD], BF16, kind="ExternalInput").ap()
    wv = nc.dram_tensor("wv", [D, GD], BF16, kind="ExternalInput").ap()
    wo = nc.dram_tensor("wo", [GD, D], BF16, kind="ExternalInput").ap()
    bq2 = nc.dram_tensor("bq2", [128, 4], FP32, kind="ExternalInput").ap()
    bk2 = nc.dram_tensor("bk2", [128, 4], FP32, kind="ExternalInput").ap()
    out = nc.dram_tensor("out", [S, D], FP32, kind="ExternalOutput").ap()

    with TileContext(nc) as tc:
        with (
            tc.tile_pool(name="cst", bufs=1) as cst_pool,
            tc.tile_pool(name="wts", bufs=3) as wts_pool,
            tc.tile_pool(name="wo", bufs=1) as wo_pool,
            tc.tile_pool(name="xt", bufs=2) as xt_pool,
            tc.tile_pool(name="kt", bufs=16) as kt_pool,
            tc.tile_pool(name="vst", bufs=16) as v_pool,
            tc.tile_pool(name="qt", bufs=8) as qt_pool,
            tc.tile_pool(name="et", bufs=6) as et_pool,
            tc.tile_pool(name="oz", bufs=5) as oz_pool,
            tc.tile_pool(name="zz", bufs=4) as zz_pool,
            tc.tile_pool(name="zt", bufs=3) as zt_pool,
            tc.tile_pool(name="ao", bufs=12) as ao_pool,
            tc.tile_pool(name="ob", bufs=4) as out_pool,
            tc.tile_pool(name="ps2", bufs=2, space="PSUM") as ps_s2,
            tc.tile_pool(name="po2", bufs=1, space="PSUM") as ps_o2,
            tc.tile_pool(name="ppj", bufs=2, space="PSUM") as ps_pj,
        ):
            # ---- constants: biases ----
            bq_t = cst_pool.tile([128, 4], FP32, tag="bq")
            bk_t = cst_pool.tile([128, 4], FP32, tag="bk")
            nc.sync.dma_start(out=bq_t[:], in_=bq2[:])
            nc.sync.dma_start(out=bk_t[:], in_=bk2[:])

            # ---- persistent SBUF tensors ----
            # kt[hp][w]: [128 (2 par x 64 hd), 512 keys] bf16
            kt_t = [[kt_pool.tile([128, W], BF16, tag="kt", name=f"kt{h}_{w}")
                     for w in range(NW)] for h in range(4)]
            # v[kc]: [128 keys, 8 heads x 65] bf16 (col 64 of each head = 1.0)
            v_t = [v_pool.tile([128, 8 * 65], BF16, tag="v", name=f"v{i}")
                   for i in range(NKC)]
            for kc in range(NKC):
                ones_ap = v_t[kc][:].rearrange(
                    "p (h e) -> p h e", e=65)[:, :, 64:65]
                nc.gpsimd.memset(ones_ap, 1.0)

            # ---- weight tiles (all persistent, bf16) ----
            # Contiguous per-chunk DMAs (strided big-AP DMAs have
            # pathological dispatch cost), distributed across the three
            # DMA-capable queues so dispatch serialization doesn't gate
            # the preamble: wk on gpsimd, wq/wv on scalar (idle during
            # the preamble), wo on sync.
            def load_w(pool, dram, n, pfx, eng):
                cols = dram.shape[1]
                big = pool.tile([128, n * cols], BF16, tag="w", name=pfx)
                for i in range(n):
                    eng.dma_start(
                        out=big[:, i * cols:(i + 1) * cols],
                        in_=dram[i * 128:(i + 1) * 128, :])
                return [big[:, i * cols:(i + 1) * cols] for i in range(n)]
            wk_t = load_w(wts_pool, wk, NDC, "wk", nc.gpsimd)
            wq_t = load_w(wts_pool, wq, NDC, "wq", nc.scalar)
            wv_t = load_w(wts_pool, wv, NDC, "wv", nc.scalar)
            wo_t = load_w(wo_pool, wo, 4, "wo", nc.sync)

            # ---- xt loading (contiguous per-chunk DMAs, sync queue) ----
            def load_xt(w):
                big = xt_pool.tile([128, NDC * W], BF16, tag="xt",
                                   name=f"xt{w}")
                for dc in range(NDC):
                    nc.sync.dma_start(
                        out=big[:, dc * W:(dc + 1) * W],
                        in_=xT[dc * DC:(dc + 1) * DC, w * W:(w + 1) * W])
                return [big[:, dc * W:(dc + 1) * W] for dc in range(NDC)]

            qt_by_w = {}

            # ---- projection micro-step builders (for weaving) ----
            # Each "step" is a closure emitting ~2 matmuls; steps of one
            # group share a PSUM tile via a cell.
            def kq_group_steps(wdst, hp, xts, which):
                # steps are (resource_label_or_None, fn): the label marks
                # the step whose emission completes that resource.
                wt = wk_t if which == "k" else wq_t
                cell = {}

                def mk(dc0):
                    def step():
                        if dc0 == 0:
                            cell["ps"] = ps_pj.tile([128, W], FP32, tag="pj",
                                                    name=f"{which}ps")
                        for dc in (dc0, dc0 + 1):
                            nc.tensor.matmul(
                                cell["ps"][:],
                                wt[dc][:, hp * 128:(hp + 1) * 128],
                                xts[dc][:], start=(dc == 0),
                                stop=(dc == NDC - 1))
                        if dc0 == NDC - 2:
                            if which == "k":
                                nc.vector.tensor_scalar_add(
                                    kt_t[hp][wdst][:], cell["ps"][:],
                                    bk_t[:, hp:hp + 1])
                            else:
                                nc.vector.tensor_scalar_add(
                                    qt_by_w[wdst][hp][:], cell["ps"][:],
                                    bq_t[:, hp:hp + 1])
                    return ((which, hp, wdst) if dc0 == NDC - 2 else None,
                            step)
                return [mk(d) for d in range(0, NDC, 2)]

            def v_group_steps(wdst, sc, xts):
                cell = {}

                def mk(dc0):
                    def step():
                        if dc0 == 0:
                            cell["ps"] = ps_pj.tile([128, W], FP32, tag="pj",
                                                    name="vps")
                        for dc in (dc0, dc0 + 1):
                            nc.tensor.matmul(
                                cell["ps"][:],
                                xts[dc][:, sc * 128:(sc + 1) * 128],
                                wv_t[dc][:], start=(dc == 0),
                                stop=(dc == NDC - 1))
                        if dc0 == NDC - 2:
                            dst = v_t[wdst * 4 + sc][:].rearrange(
                                "p (h e) -> p h e", e=65)[:, :, 0:64]
                            src = cell["ps"][:].rearrange(
                                "p (h e) -> p h e", e=64)
                            nc.vector.tensor_copy(dst, src)
                    return (("v", wdst * 4 + sc) if dc0 == NDC - 2 else None,
                            step)
                return [mk(d) for d in range(0, NDC, 2)]

            def o_group_steps(wsrc, qs, dcol, aos):
                cell = {}

                def mk(hc0):
                    def step():
                        if hc0 == 0:
                            cell["ps"] = ps_pj.tile([128, W], FP32, tag="pj",
                                                    name="ops")
                        for hc in (hc0, hc0 + 1):
                            nc.tensor.matmul(
                                cell["ps"][:],
                                aos[hc][:, qs * 128:(qs + 1) * 128],
                                wo_t[hc][:, dcol * 512:(dcol + 1) * 512],
                                start=(hc == 0), stop=(hc == 3))
                        if hc0 == 2:
                            ot = out_pool.tile([128, W], FP32, tag="ob")
                            # alternate output DMA queues; in the tail
                            # window (ACT idle) also alternate eviction
                            # engines so evictions and transfers pipeline
                            if (qs + dcol) % 2 == 0:
                                nc.vector.tensor_copy(ot[:], cell["ps"][:])
                                eng = nc.sync
                            else:
                                if wsrc == NW - 1:
                                    nc.scalar.copy(ot[:], cell["ps"][:])
                                else:
                                    nc.vector.tensor_copy(ot[:],
                                                          cell["ps"][:])
                                eng = nc.gpsimd
                            eng.dma_start(
                                out=out[wsrc * W + qs * 128:
                                        wsrc * W + (qs + 1) * 128,
                                        dcol * 512:(dcol + 1) * 512],
                                in_=ot[:])
                    return (None, step)
                return [mk(h) for h in range(0, 4, 2)]

            # ---- attention emitters ----
            def emit_S(w, hp, kcs, qts):
                """Score matmuls + exp (+ diag mask) for a batch of chunks.
                Returns {kc: et_tile}."""
                ets = {}
                for kc in kcs:
                    j = kc - 4 * w
                    lo = max(j, 0) * 128
                    s2 = ps_s2.tile([128, 1024], FP32, tag="s2", name="s2")
                    et = et_pool.tile([128, 1024], BF16, tag="et")
                    for par in range(2):
                        nc.tensor.matmul(
                            s2[:, par * 512 + lo:par * 512 + 512],
                            kt_t[hp][kc // 4][par * 64:(par + 1) * 64,
                                              (kc % 4) * 128:(kc % 4 + 1) * 128],
                            qts[hp][par * 64:(par + 1) * 64, lo:W],
                            start=True, stop=True)
                    if lo == 0:
                        nc.scalar.activation(et[:], s2[:], EXP,
                                             bias=0.0, scale=SCALE)
                    else:
                        sv = s2[:].rearrange("p (two n) -> p two n",
                                             two=2)[:, :, lo:512]
                        ev = et[:].rearrange("p (two n) -> p two n",
                                             two=2)[:, :, lo:512]
                        nc.scalar.activation(ev, sv, EXP,
                                             bias=0.0, scale=SCALE)
                    if j >= 0:
                        seg = et[:].rearrange("p (two n) -> p two n",
                                              two=2)[:, :, lo:lo + 128]
                        nc.gpsimd.affine_select(
                            out=seg, in_=seg,
                            compare_op=mybir.AluOpType.is_ge,
                            fill=0.0, base=0, pattern=[[0, 2], [1, 128]],
                            channel_multiplier=-1)
                    ets[kc] = et
                return ets

            def emit_V(w, hp, o2, kcs, ets, nkc):
                for kc in kcs:
                    j = kc - 4 * w
                    lo = max(j, 0) * 128
                    for par in range(2):
                        h = 2 * hp + par
                        nc.tensor.matmul(
                            o2[0:65, par * 512 + lo:par * 512 + 512],
                            v_t[kc][:, h * 65:(h + 1) * 65],
                            ets[kc][:, par * 512 + lo:par * 512 + 512],
                            start=(kc == 0), stop=(kc == nkc - 1))

            def emit_norm_hp(w, hp, o2, aos):
                # Evict o2 (incl. Z row 64) to SBUF fast, freeing PSUM.
                # Z handling: scatter the [1,1024] Z row into 16 partitions
                # (reciprocal then runs on 16 lanes: ~0.55us vs 8.5us),
                # gather back, broadcast per par half, two muls -> bf16 ao.
                oz = oz_pool.tile([128, 1024], FP32, tag="oz",
                                  name=f"oz{w}_{hp}")
                nc.vector.tensor_copy(oz[0:65, :], o2[0:65, :])
                zzT = zt_pool.tile([16, 64], FP32, tag="zzT")
                nc.sync.dma_start(
                    out=zzT[:],
                    in_=oz[64:65, :].rearrange("p (f n) -> p f n", f=16))
                zrT = zt_pool.tile([16, 64], FP32, tag="zrT")
                nc.vector.reciprocal(zrT[:], zzT[:])
                zr = zz_pool.tile([1, 1024], FP32, tag="zr")
                nc.sync.dma_start(
                    out=zr[:].rearrange("p (f n) -> p f n", f=16),
                    in_=zrT[:])
                for par in range(2):
                    zb = zz_pool.tile([64, 512], FP32, tag="zb")
                    nc.gpsimd.partition_broadcast(
                        zb[:], zr[0:1, par * 512:(par + 1) * 512])
                    nc.vector.tensor_mul(
                        aos[hp][par * 64:(par + 1) * 64, :],
                        oz[0:64, par * 512:(par + 1) * 512],
                        zb[:])
                return oz

            # ---- minimal preamble: only what window-0/hp0 attention
            # needs up front (K/Q for hp0, all V chunks of window 0);
            # the remaining hp1-3 K/Q projections weave into window 0.
            xts0 = load_xt(0)
            qt_by_w[0] = [qt_pool.tile([128, W], BF16, tag="qt",
                                       name=f"qt0_{h}") for h in range(4)]
            emitted_res = set()
            pre_steps = kq_group_steps(0, 0, xts0, "k")
            pre_steps += kq_group_steps(0, 0, xts0, "q")
            for lab, st in pre_steps:
                st()
                if lab:
                    emitted_res.add(lab)
            w0_rest = []
            for sc in range(4):
                w0_rest += v_group_steps(0, sc, xts0)
            for hp in range(1, 4):
                w0_rest += kq_group_steps(0, hp, xts0, "k")
                w0_rest += kq_group_steps(0, hp, xts0, "q")

            ao_by_w = {}
            vcarry = []
            last_oz = None

            # ---- main loop over query windows ----
            for w in range(NW):
                nkc = 4 * (w + 1)
                qts = qt_by_w[w]
                ao_by_w[w] = [ao_pool.tile([128, W], BF16, tag="ao",
                                           name=f"ao{w}_{h}")
                              for h in range(4)]

                # build this window's proj-step queue; V-projection groups
                # for window w+1 carry over to the FRONT of window w+1's
                # queue (they are needed later than K/Q), relieving the
                # PE-heavy early windows.
                steps = []
                steps += vcarry
                vcarry = []
                if w == 0:
                    steps += w0_rest
                if w + 1 < NW:
                    xts = load_xt(w + 1)
                    qt_by_w[w + 1] = [qt_pool.tile([128, W], BF16, tag="qt",
                                                   name=f"qt{w + 1}_{h}")
                                      for h in range(4)]
                    for hp in range(4):
                        steps += kq_group_steps(w + 1, hp, xts, "k")
                        steps += kq_group_steps(w + 1, hp, xts, "q")
                    for sc in range(4):
                        vcarry += v_group_steps(w + 1, sc, xts)
                # O-projections run two windows late (w-2) except w3,
                # which also absorbs w2's: the late windows are ACT-bound
                # with PE slack, the early ones are PE-bound.
                osrcs = []
                if w >= 2:
                    osrcs.append(w - 2)
                if w == NW - 1:
                    osrcs.append(w - 1)
                for wsrc in osrcs:
                    for qs in range(4):
                        for dcol in range(2):
                            steps += o_group_steps(wsrc, qs, dcol,
                                                   ao_by_w[wsrc])
                # consume steps evenly across attention batches; `need`
                # force-pops until a resource's producer has been emitted
                # (program order must respect producer-before-consumer).
                nbatch = (nkc + 1) // 2 * 4          # batches this window
                sq = list(steps)
                popped = [0]
                bdone = [0]

                def pop_one():
                    lab, st = sq[popped[0]]
                    st()
                    if lab:
                        emitted_res.add(lab)
                    popped[0] += 1

                def pop_steps():
                    bdone[0] += 1
                    want = min(len(sq),
                               (len(sq) * bdone[0]) // nbatch + 2)
                    while popped[0] < want:
                        pop_one()

                def need(lab):
                    while lab not in emitted_res:
                        assert popped[0] < len(sq), f"missing producer {lab}"
                        pop_one()

                for hp in range(4):
                    need(("k", hp, w))
                    need(("q", hp, w))
                    o2 = ps_o2.tile([128, 1024], FP32, tag="o2",
                                    name=f"o2_{w}_{hp}")
                    pending = None
                    for kc0 in range(0, nkc, 2):
                        kcs = list(range(kc0, min(kc0 + 2, nkc)))
                        ets = emit_S(w, hp, kcs, qts)
                        pop_steps()
                        if pending is not None:
                            for kc in pending[0]:
                                need(("v", kc))
                            emit_V(w, hp, o2, pending[0], pending[1], nkc)
                        pending = (kcs, ets)
                    for kc in pending[0]:
                        need(("v", kc))
                    emit_V(w, hp, o2, pending[0], pending[1], nkc)
                    last_oz = emit_norm_hp(w, hp, o2, ao_by_w[w])
                # leftover proj steps (rounding)
                while popped[0] < len(sq):
                    pop_one()

            # ---- tail: output projection for the last window ----
            # Dummy accumulation reading the last oz keeps the PE busy
            # across the hp3 norm chain (scatter/recip/gather/bcast/mul)
            # so HAM stays at full clock for the tail matmuls. The oz
            # data dependency pins these into the gap.
            ozb = oz_pool.tile([128, 1024], BF16, tag="ozb", name="ozb")
            nc.vector.tensor_copy(ozb[0:64, 0:512], last_oz[0:64, 0:512])
            warm = ps_pj.tile([128, W], FP32, tag="pj", name="warm")
            for i in range(10):
                nc.tensor.matmul(warm[0:64, :], ozb[0:64, 0:64],
                                 ozb[0:64, 0:512],
                                 start=(i == 0), stop=(i == 9))
            for qs in range(4):
                for dcol in range(2):
                    for _lab, st in o_group_steps(NW - 1, qs, dcol,
                                                  ao_by_w[NW - 1]):
                        st()

    nc.compile()
    return nc


def _get_program():
    if "nc" not in _CACHE:
        _CACHE["nc"] = _build_program()
    return _CACHE["nc"]


def _install_ntff_hook():
    """The agent image's antenv lacks axon_hooks; shim it and register the
    ctypes NTFF profiling hook so trace=True yields exec_time_ns."""
    import sys, types
    if "antenv.axon_hooks" in sys.modules:
        return
    try:
        import antenv
        mod = types.ModuleType("antenv.axon_hooks")
        _h = [None]
        mod.set_axon_ntff_profile_hook = lambda h: _h.__setitem__(0, h)
        mod.get_axon_ntff_profile_hook = lambda: _h[0]
        sys.modules["antenv.axon_hooks"] = mod
        antenv.axon_hooks = mod
        from trn_agent_boot.trn_boot import _ntff_profile_via_ctypes
        mod.set_axon_ntff_profile_hook(
            _ntff_profile_via_ctypes("/opt/axon/libaxon_pjrt.so"))
    except Exception as e:  # degrade: run without tracing
        print(f"NTFF hook install failed ({e}); tracing disabled")


def _run(inputs, trace=False):
    bf = ml_dtypes.bfloat16
    x = np.asarray(inputs["x"], dtype=np.float32)
    Wq = np.asarray(inputs["Wq"], dtype=np.float32)
    Wk = np.asarray(inputs["Wk"], dtype=np.float32)
    Wv = np.asarray(inputs["Wv"], dtype=np.float32)
    Wo = np.asarray(inputs["Wo"], dtype=np.float32)
    bq = np.asarray(inputs["bq"], dtype=np.float32)
    bk = np.asarray(inputs["bk"], dtype=np.float32)
    bv = np.asarray(inputs["bv"], dtype=np.float32)
    bo = np.asarray(inputs["bo"], dtype=np.float32)

    if trace:
        _install_ntff_hook()
    nc = _get_program()
    in_maps = []
    for c in range(NCORES):
        b, g = divmod(c, 2)
        sl = slice(g * GD, (g + 1) * GD)
        in_maps.append({
            "xT": np.ascontiguousarray(x[b].T).astype(bf),
            "wq": np.ascontiguousarray(Wq[:, sl]).astype(bf),
            "wk": np.ascontiguousarray(Wk[:, sl]).astype(bf),
            "wv": np.ascontiguousarray(Wv[:, sl]).astype(bf),
            "wo": np.ascontiguousarray(Wo[sl, :]).astype(bf),
            "bq2": np.ascontiguousarray(bq[sl].reshape(4, 128).T),
            "bk2": np.ascontiguousarray(bk[sl].reshape(4, 128).T),
        })
    res = run_bass_kernel_spmd(nc, in_maps, list(range(NCORES)), trace=trace)
    outp = np.empty((B, S, D), dtype=np.float32)
    # bv correction: attention rows sum to 1, so x @ Wv + bv contributes
    # attn@V + bv per row; bv flows through Wo as a constant row vector.
    corr = (bv @ Wo + bo).astype(np.float32)
    for b in range(B):
        outp[b] = res.results[2 * b]["out"] + res.results[2 * b + 1]["out"] + corr
    return outp, res


def kernel(**inputs):
    outp, _ = _run(inputs, trace=False)
    return outp


def kernel_traced(**inputs):
    outp, res = _run(inputs, trace=True)
    return outp, res
